# revision 37
# baseline (speedup 1.0000x reference)
"""DEQ block (Anderson acceleration, 6 iters, m=3) on 8 trn2 NeuronCores.

Data-parallel over batch: each core gets 512 of 4096 samples; W_z/W_x/b
replicated.  All 512 samples (4 m-tiles of 128) stay SBUF resident in
fp16, so the pointwise/dot/solve work of one n-slice overlaps the PE
matmuls of the next.  W_z is cast to fp16 in DRAM once (SWDGE cast-DMA)
and streamed per iteration in (n,k) consumption order through a small
SBUF ring; x/W_x stream through a deep dual-queue (SP+ACT HWDGE) ring.

Per iteration i (z update, sample-major state):
  f   = tanh(z @ W_z + xwx)            PE (+identity-matmul xwx add) + ACT
  g   = f - z ; u = beta*g + z         DVE STT, fp16
  i<3:  z' = u  (buffer alias, no copy)
  i>=3: 2x2 regularized Anderson solve from 3 fresh dots
        P=<g,g> (ACT square+accum), Q1=<g,g1>, Q2=<g,g2> (DVE STT),
        gram history terms reused from previous iterations' P/Q1;
        z' = s0*u + gamma1*u1 + gamma2*u2  (ACT scale + 2 DVE STT)
"""

import os
import sys

sys.path.insert(0, "/opt/trn_rl_repo")

import numpy as np
from contextlib import ExitStack

import concourse.bass as bass
import concourse.tile as tile
from concourse import bacc, mybir, masks
from concourse import bass_utils

F32 = mybir.dt.float32
F16 = mybir.dt.float16
ALU = mybir.AluOpType
ACTF = mybir.ActivationFunctionType

B, D = 4096, 2048
NCORES = 8
BC = B // NCORES          # 512 samples per core
MT = BC // 128            # 4 m-tiles
KT = D // 128             # 16 k-tiles
NT = D // 512             # 4 n-slices
MAX_ITER, MAND = 6, 3
BETA, LAM = 0.8, 1e-4

_CACHE = {}

NITER = int(os.environ.get("K_NITER", str(MAX_ITER)))
WZRING = int(os.environ.get("K_WZRING", "12"))


def _build():
    nc = bacc.Bacc("TRN2", target_bir_lowering=False, debug=False,
                   num_devices=NCORES)

    x_d = nc.dram_tensor("x", [BC, D], F32, kind="ExternalInput").ap()
    wz_d = nc.dram_tensor("W_z", [D, D], F32, kind="ExternalInput").ap()
    wx_d = nc.dram_tensor("W_x", [D, D], F32, kind="ExternalInput").ap()
    b_d = nc.dram_tensor("b", [D], F32, kind="ExternalInput").ap()
    out_d = nc.dram_tensor("z_out", [BC, D], F32, kind="ExternalOutput").ap()
    # W_z staged fp16 in DRAM (cast once, streamed every iteration)
    wzst_d = nc.dram_tensor("wz_f16", [D, D], F16, kind="Internal").ap()

    with tile.TileContext(nc) as tc, ExitStack() as ctx:
        # ---------------- pools ----------------
        state = ctx.enter_context(tc.tile_pool(name="state", bufs=1))

        def persist(shape, nm, dt=F16):
            return state.tile(shape, dt, tag=nm, name=nm)

        zbuf = [persist([128, D], f"zbuf{m}") for m in range(MT)]
        gsl = [[persist([128, D], f"g{j}_{m}") for m in range(MT)]
               for j in range(3)]
        usl = [[persist([128, D], f"u{j}_{m}") for m in range(MT)]
               for j in range(3)]
        xwx = [persist([128, D], f"xwx{m}") for m in range(MT)]
        zt = [persist([128, D], f"zt{m}") for m in range(MT)]
        bsl = [persist([128, 512], f"bsl{n}", F32) for n in range(NT)]
        pdump = persist([128, 512], "pdump", F32)
        qdump = persist([128, 512], "qdump", F32)
        ident = persist([128, 128], "ident", F32)
        # P/Q1 dot history rings, m-batched [128, MT] (3 generations)
        Ph = [persist([128, MT], f"P{j}", F32) for j in range(3)]
        Q1h = [persist([128, MT], f"Q1_{j}", F32) for j in range(3)]

        wpool = ctx.enter_context(tc.tile_pool(name="wstream", bufs=10))
        diags = ctx.enter_context(tc.tile_pool(name="diags", bufs=16))
        wxf16 = ctx.enter_context(tc.tile_pool(name="wxf16", bufs=8))
        wzr = ctx.enter_context(tc.tile_pool(name="wzring", bufs=WZRING))
        dots = ctx.enter_context(tc.tile_pool(name="dots", bufs=96))
        typs = ctx.enter_context(tc.tile_pool(name="tpsum", bufs=3, space="PSUM"))
        yps = ctx.enter_context(tc.tile_pool(name="ypsum", bufs=5, space="PSUM"))

        masks.make_identity(nc, ident[:])
        identh = persist([128, 128], "identh")
        nc.vector.tensor_copy(identh[:], ident[:])
        rid = ident[:]          # fp32, rhs for fp32 transposes
        ridh = identh[:]        # fp16, lhsT of the xwx identity-matmul

        def stt(out, in0, scalar, in1, op0, op1, **kw):
            nc.vector.scalar_tensor_tensor(
                out=out, in0=in0, scalar=scalar, in1=in1, op0=op0, op1=op1,
                **kw)

        # b first (tiny, on the sync queue) so the xwx drains never wait
        b2d = b_d.rearrange("(p n) -> p n", p=1)
        for n in range(NT):
            b1 = wpool.tile([1, 512], F32, tag="w", name=f"b1_{n}")
            nc.sync.dma_start(b1[:], b2d[:, n * 512:(n + 1) * 512])
            nc.gpsimd.partition_broadcast(bsl[n][:], b1[:])

        dmaq = [nc.sync, nc.scalar]   # two HWDGE queues, round-robin
        qi = 0

        def stream_dma(dst, src):
            nonlocal qi
            dmaq[qi % 2].dma_start(dst, src)
            qi += 1

        # xT backing: 16 transposed-x k-tiles [128, 512] fp16 live inside
        # the (not yet used) u0 tiles during phase 0.
        def xt_sl(k, q):
            return usl[0][k // 4][:, (k % 4) * 512 + q * 128:
                                  (k % 4) * 512 + (q + 1) * 128]

        # ---------------- phase 0: xwx for all 4 quarter-tiles ----------------
        for q in range(4):
            xs = []
            for h4 in range(4):
                xst = wpool.tile([128, 512], F32, tag="w", name=f"xst{q}_{h4}")
                stream_dma(xst[:], x_d[q * 128:(q + 1) * 128,
                                       h4 * 512:(h4 + 1) * 512])
                xs.append(xst)
            for kb in range(4):
                tp = typs.tile([128, 512], F32, tag="tp", name=f"xtp{q}_{kb}")
                for j in range(4):
                    k = kb * 4 + j
                    nc.tensor.transpose(
                        tp[:, j * 128:(j + 1) * 128],
                        xs[k // 4][:, (k % 4) * 128:(k % 4 + 1) * 128], rid)
                for j in range(4):
                    k = kb * 4 + j
                    nc.vector.tensor_copy(xt_sl(k, q),
                                          tp[:, j * 128:(j + 1) * 128])

        for n in range(NT):
            ps = [yps.tile([128, 512], F32, tag="yp", name=f"xwps{n}_{q}")
                  for q in range(4)]
            for k in range(KT):
                wt32 = wpool.tile([128, 512], F32, tag="w", name=f"wx{n}_{k}")
                # split by k-range (not parity) so each queue delivers a
                # contiguous run in consumption order
                eng = nc.sync if k < KT // 2 else nc.scalar
                eng.dma_start(wt32[:], wx_d[k * 128:(k + 1) * 128,
                                            n * 512:(n + 1) * 512])
                wt = wxf16.tile([128, 512], F16, tag="wx16", name=f"wxh{n}_{k}")
                nc.vector.tensor_copy(wt[:], wt32[:])
                for q in range(4):
                    nc.tensor.matmul(ps[q][:], xt_sl(k, q), wt[:],
                                     start=(k == 0), stop=(k == KT - 1))
            for q in range(4):
                stt(xwx[q][:, n * 512:(n + 1) * 512], ps[q][:], 1.0,
                    bsl[n][:], ALU.mult, ALU.add)

        # ---------------- iterations ----------------
        hist_q2 = {}

        # iteration 0: z=0 -> g0 = tanh(xwx), u0 = beta*g0, z1 aliases u0
        for m in range(MT):
            nc.scalar.activation(gsl[0][m][:], xwx[m][:], ACTF.Tanh)
            nc.vector.tensor_scalar_mul(usl[0][m][:], gsl[0][m][:], BETA)

        for i in range(1, NITER):
            gi, ui = gsl[i % 3], usl[i % 3]
            g1, g2 = gsl[(i - 1) % 3], gsl[(i - 2) % 3]
            u1, u2 = usl[(i - 1) % 3], usl[(i - 2) % 3]
            zc = usl[i - 1] if i <= 3 else zbuf  # current z (alias)

            def emit_transposes(kb):
                # transpose z kb-block into lhsT k-tiles (PE) + psum->sbuf
                # fp16 (ACT); interleaved with the n=0 matmul sub-blocks so
                # PE restarts as soon as the first z' n-chunks land
                for m in range(MT):
                    tp = typs.tile([128, 512], F16, tag="tp",
                                   name=f"tp{i}_{m}_{kb}")
                    for j in range(4):
                        k = kb * 4 + j
                        nc.tensor.transpose(
                            tp[:, j * 128:(j + 1) * 128],
                            zc[m][:, k * 128:(k + 1) * 128], ridh)
                    nc.vector.tensor_copy(
                        zt[m][:, kb * 512:(kb + 1) * 512], tp[:])

            # W_z chunk stream, (n,k) consumption order.  Iteration 1
            # streams the fp32 original (HWDGE), casts to fp16 on DVE and
            # writes the fp16 copy back to DRAM; iterations 2+ stream the
            # staged fp16 at half the bytes.  No phase-0 SWDGE bulk
            # traffic starving the x/W_x stream.
            wchunk = {}
            for n in range(NT):
                for k in range(KT):
                    wt = wzr.tile([128, 512], F16, tag="wz",
                                  name=f"wz{i}_{n}_{k}")
                    src2d = slice(k * 128, (k + 1) * 128)
                    csl = slice(n * 512, (n + 1) * 512)
                    rd = nc.sync if k < KT // 2 else nc.scalar
                    if i == 1:
                        w32 = wpool.tile([128, 512], F32, tag="w",
                                         name=f"wz32_{n}_{k}")
                        rd.dma_start(w32[:], wz_d[src2d, csl])
                        nc.vector.tensor_copy(wt[:], w32[:])
                        wb = nc.scalar if k < KT // 2 else nc.sync
                        wb.dma_start(wzst_d[src2d, csl], wt[:])
                    else:
                        rd.dma_start(wt[:], wzst_d[src2d, csl])
                    wchunk[n, k] = wt

            # matmul + xwx add + tanh + pointwise + dot chunks, n-major;
            # chunk dot accumulators are m-batched: column m*4+n
            pca = dots.tile([128, 16], F32, tag="d", name=f"pca{i}")
            q1a = dots.tile([128, 16], F32, tag="d", name=f"q1a{i}")
            q2a = dots.tile([128, 16], F32, tag="d", name=f"q2a{i}")
            for n in range(NT):
                sl = slice(n * 512, (n + 1) * 512)
                ps = [yps.tile([128, 512], F32, tag="yp", name=f"yp{i}_{n}_{m}")
                      for m in range(MT)]
                for k in range(KT):
                    if n == 0 and k % 4 == 0:
                        emit_transposes(k // 4)
                    wsl = wchunk[n, k][:]
                    for m in range(MT):
                        nc.tensor.matmul(ps[m][:],
                                         zt[m][:, k * 128:(k + 1) * 128], wsl,
                                         start=(k == 0), stop=(k == KT - 1))
                for m in range(MT):
                    # xwx add in-place in PSUM on DVE (saves a PE matmul)
                    stt(ps[m][:], ps[m][:], 1.0, xwx[m][:, sl],
                        ALU.mult, ALU.add)
                for m in range(MT):
                    c = m * 4 + n
                    # f into the g slot (fp16), then g = f - z, u = b*g + z
                    nc.scalar.activation(gi[m][:, sl], ps[m][:], ACTF.Tanh)
                    stt(gi[m][:, sl], gi[m][:, sl], 1.0, zc[m][:, sl],
                        ALU.mult, ALU.subtract)
                    stt(ui[m][:, sl], gi[m][:, sl], BETA, zc[m][:, sl],
                        ALU.mult, ALU.add)
                    # dot chunks
                    nc.scalar.activation(pdump[:], gi[m][:, sl], ACTF.Square,
                                         accum_out=pca[:, c:c + 1])
                    if i >= 2:
                        stt(qdump[:], gi[m][:, sl], 1.0, g1[m][:, sl],
                            ALU.mult, ALU.mult,
                            accum_out=q1a[:, c:c + 1])
                    if i >= 3:
                        stt(qdump[:], gi[m][:, sl], 1.0, g2[m][:, sl],
                            ALU.mult, ALU.mult,
                            accum_out=q2a[:, c:c + 1])

            # m-batched reduces + single [128, MT] solve chain (all DVE)
            def red(dst, src):
                nc.vector.tensor_reduce(
                    dst, src.rearrange("p (m n) -> p m n", n=4),
                    mybir.AxisListType.X, ALU.add)

            red(Ph[i % 3][:], pca[:])
            if i >= 2:
                red(Q1h[i % 3][:], q1a[:])
            if i >= 3:
                q2 = dots.tile([128, MT], F32, tag="d", name=f"q2_{i}")
                red(q2[:], q2a[:])

                P = Ph[i % 3][:]
                Q1 = Q1h[i % 3][:]
                Q2 = q2[:]
                S11 = Ph[(i - 1) % 3][:]
                S12 = Q1h[(i - 1) % 3][:]
                S22 = Ph[(i - 2) % 3][:]

                def tnew(nm):
                    return dots.tile([128, MT], F32, tag="d",
                                     name=f"{nm}_{i}")[:]

                def tt(out, a, b, op):
                    nc.vector.tensor_tensor(out, a, b, op)

                r0 = tnew("r0"); tt(r0, P, Q1, ALU.subtract)
                r1 = tnew("r1"); tt(r1, P, Q2, ALU.subtract)
                a1 = tnew("a1"); stt(a1, Q1, -2.0, S11, ALU.mult, ALU.add)
                av = tnew("av"); stt(av, a1, LAM, P, ALU.add, ALU.add)
                d1 = tnew("d1"); stt(d1, Q2, -2.0, S22, ALU.mult, ALU.add)
                dv = tnew("dv"); stt(dv, d1, LAM, P, ALU.add, ALU.add)
                b1 = tnew("b1"); stt(b1, Q2, -1.0, S12, ALU.mult, ALU.add)
                bv = tnew("bv"); tt(bv, b1, r0, ALU.add)
                t4 = tnew("t4"); tt(t4, av, dv, ALU.mult)
                t5 = tnew("t5"); tt(t5, bv, bv, ALU.mult)
                det = tnew("det")
                stt(det, t4, 1e-8, t5, ALU.add, ALU.subtract)
                idet = tnew("idet"); nc.vector.reciprocal(idet, det)
                g1a = tnew("g1a"); tt(g1a, dv, r0, ALU.mult)
                g1b = tnew("g1b"); tt(g1b, bv, r1, ALU.mult)
                g1c = tnew("g1c"); tt(g1c, g1a, g1b, ALU.subtract)
                gam1 = tnew("gam1"); tt(gam1, g1c, idet, ALU.mult)
                g2a = tnew("g2a"); tt(g2a, av, r1, ALU.mult)
                g2b = tnew("g2b"); tt(g2b, bv, r0, ALU.mult)
                g2c = tnew("g2c"); tt(g2c, g2a, g2b, ALU.subtract)
                gam2 = tnew("gam2"); tt(gam2, g2c, idet, ALU.mult)
                s0a = tnew("s0a")
                stt(s0a, gam1, -1.0, gam2, ALU.mult, ALU.subtract)
                s0 = tnew("s0")
                nc.vector.tensor_scalar(s0, s0a, 1.0, None, ALU.add)

                # z' = s0*u + gam1*u1 + gam2*u2 computed on PE as three
                # accumulating diag(c) @ u matmuls per n-chunk (the
                # per-sample scalars become diagonal lhsT tiles), drained
                # by ACT -- keeps DVE off the iteration-boundary critical
                # path entirely
                dg = []
                for m in range(MT):
                    row = []
                    for nm_, coef in (("s0", s0), ("g1", gam1), ("g2", gam2)):
                        dgt = diags.tile([128, 128], F16, tag="dg",
                                         name=f"dg{nm_}_{i}_{m}")
                        nc.vector.tensor_scalar(dgt[:], identh[:],
                                                coef[:, m:m + 1], None,
                                                ALU.mult)
                        row.append(dgt)
                    dg.append(row)
                for n in range(NT):
                    sl = slice(n * 512, (n + 1) * 512)
                    for m in range(MT):
                        tpz = yps.tile([128, 512], F32, tag="yp",
                                       name=f"zp{i}_{n}_{m}")
                        for j, usrc in enumerate((ui, u1, u2)):
                            nc.tensor.matmul(tpz[:], dg[m][j][:],
                                             usrc[m][:, sl],
                                             start=(j == 0), stop=(j == 2))
                        nc.vector.tensor_copy(zbuf[m][:, sl], tpz[:])
                        if i == NITER - 1:
                            # drain out via DVE f32 cast + dual-queue HWDGE
                            # (Pool SWDGE is too slow for the 4MB tail)
                            ob = wpool.tile([128, 512], F32, tag="w",
                                            name=f"ob{m}_{n}")
                            nc.vector.tensor_copy(ob[:], zbuf[m][:, sl])
                            eng = nc.sync if (m + n) % 2 == 0 else nc.scalar
                            eng.dma_start(out_d[m * 128:(m + 1) * 128, sl],
                                          ob[:])

            if i == NITER - 1 and i < 3:
                for m in range(MT):
                    nc.gpsimd.dma_start(out_d[m * 128:(m + 1) * 128, :],
                                        ui[m][:])

    nc.compile()
    return nc


def kernel(x_input, W_z, W_x, b):
    x_input = np.ascontiguousarray(x_input, dtype=np.float32)
    W_z = np.ascontiguousarray(W_z, dtype=np.float32)
    W_x = np.ascontiguousarray(W_x, dtype=np.float32)
    b = np.ascontiguousarray(b, dtype=np.float32)

    if "nc" not in _CACHE:
        _CACHE["nc"] = _build()
    nc = _CACHE["nc"]

    in_maps = [{
        "x": x_input[i * BC:(i + 1) * BC],
        "W_z": W_z, "W_x": W_x, "b": b,
    } for i in range(NCORES)]

    res = bass_utils.run_bass_kernel_spmd(nc, in_maps,
                                          core_ids=list(range(NCORES)))
    out = np.concatenate([res.results[i]["z_out"] for i in range(NCORES)],
                         axis=0)
    return out.astype(np.float32)


def profile_run(x_input, W_z, W_x, b):
    """Profiled run (NTFF trace) returning HW exec time in ns."""
    x_input = np.ascontiguousarray(x_input, dtype=np.float32)
    W_z = np.ascontiguousarray(W_z, dtype=np.float32)
    W_x = np.ascontiguousarray(W_x, dtype=np.float32)
    b = np.ascontiguousarray(b, dtype=np.float32)

    if "nc" not in _CACHE:
        _CACHE["nc"] = _build()
    nc = _CACHE["nc"]

    in_maps = [{
        "x": x_input[i * BC:(i + 1) * BC],
        "W_z": W_z, "W_x": W_x, "b": b,
    } for i in range(NCORES)]

    import glob as globmod
    import tempfile

    from trn_agent_boot.trn_boot import _ntff_profile_via_ctypes
    from concourse import bass2jax
    from concourse._compat import FishPath
    import gauge.profiler

    hook = _ntff_profile_via_ctypes("/opt/axon/libaxon_pjrt.so")
    if hook is None:
        print("no NTFF profile hook available")
        return None
    neff_dir = tempfile.mkdtemp(prefix="bass_prof_")
    with hook(neff_dir, [0]):
        bass2jax.run_bass_via_pjrt(nc, in_maps, n_cores=NCORES)
    ntffs = globmod.glob(os.path.join(neff_dir, "*_body*.ntff"))
    if not ntffs:
        print("no ntff files found:", sorted(os.listdir(neff_dir)))
        return None
    profile = gauge.profiler.Profile(
        profile_path=FishPath(neff_dir),
        kernel_dev_mode=True,
        profile_on_exit=False,
        bass_kernel=nc.m,
        offline_processing=True,
        fname="*_body*",
    )
    pr = profile.to_perfetto(model_index=(0,))
    if not pr:
        print("to_perfetto produced no results")
        return None
    print(f"trace: {pr[0].trace_path}")
    print(f"profile artifacts in {neff_dir}")
    return pr[0].exec_time_ns
